# revision 21
# baseline (speedup 1.0000x reference)
"""Multi-head attention (B=2, S=2048, D=1024, H=16, causal) on 8 TRN2 cores.

Sharding: tensor-parallel over heads — 2 heads per core. Each core projects
q/k/v for its head slice (contraction over the full model dim), runs causal
attention for its (batch, head) pairs, and computes a partial output
projection over its 128 columns of w_o. The host sums the 8 partials.

All matmuls run in bf16 with fp32 PSUM accumulation. Inputs are cast and
pre-transposed on the host so activations arrive as [dim, token] (the layout
the tensor engine needs for contraction over dims); weights arrive pre-sliced
per core.
"""
import sys
from contextlib import ExitStack

for _p in ("/opt/trn_rl_repo", "/root/.axon_site/_ro/trn_rl_repo"):
    if _p not in sys.path:
        sys.path.insert(0, _p)

import numpy as np
import ml_dtypes

B, S, D, H = 2, 2048, 1024, 16
DK = 64          # head dim
NCORES = 8
HPC = H // NCORES  # heads per core
DL = HPC * DK      # local feature dim per core (128)
BS = B * S         # 4096 tokens, batch-major columns
NB = S // 128      # 16 key/query blocks per batch
QT = 512           # q-tile width
BF16 = ml_dtypes.bfloat16

_compiled = {}


def _build():
    import concourse.mybir as mybir
    import concourse.tile as tile
    from concourse import bacc

    F32 = mybir.dt.float32
    BF = mybir.dt.bfloat16
    EXP = mybir.ActivationFunctionType.Exp
    MULT = mybir.AluOpType.mult

    nc = bacc.Bacc("TRN2", target_bir_lowering=False, debug=False)

    qT = nc.dram_tensor("qT", [D, BS], BF, kind="ExternalInput")
    kT = nc.dram_tensor("kT", [D, BS], BF, kind="ExternalInput")
    vT = nc.dram_tensor("vT", [D, BS], BF, kind="ExternalInput")
    wqT = nc.dram_tensor("wqT", [D, DL], BF, kind="ExternalInput")
    wkT = nc.dram_tensor("wkT", [D, DL], BF, kind="ExternalInput")
    wvT = nc.dram_tensor("wvT", [D, DL], BF, kind="ExternalInput")
    bq = nc.dram_tensor("bq", [DL, 1], F32, kind="ExternalInput")
    bk = nc.dram_tensor("bk", [DL, 1], F32, kind="ExternalInput")
    bvr = nc.dram_tensor("bvr", [1, DL], BF, kind="ExternalInput")
    woT = nc.dram_tensor("woT", [DL, D], BF, kind="ExternalInput")
    out = nc.dram_tensor("out", [BS, D], F32, kind="ExternalOutput")

    NCH = D // 128  # contraction chunks

    with tile.TileContext(nc) as tc, ExitStack() as ctx:
        const = ctx.enter_context(tc.tile_pool(name="const", bufs=1))
        wpool = ctx.enter_context(tc.tile_pool(name="wpool", bufs=1))
        persist = ctx.enter_context(tc.tile_pool(name="persist", bufs=1))
        inp = ctx.enter_context(tc.tile_pool(name="inp", bufs=6))
        vinp = ctx.enter_context(tc.tile_pool(name="vinp", bufs=10))
        work = ctx.enter_context(tc.tile_pool(name="work", bufs=4))
        evict = ctx.enter_context(tc.tile_pool(name="evict", bufs=2))
        mm_ps = ctx.enter_context(tc.tile_pool(name="mm_ps", bufs=2, space="PSUM"))
        pv_ps_pool = ctx.enter_context(tc.tile_pool(name="pv_ps", bufs=3, space="PSUM"))
        rb_ps_pool = ctx.enter_context(tc.tile_pool(name="rb_ps", bufs=1, space="PSUM"))

        # --- constants ---
        tri = const.tile([128, 128], BF)  # tri[k, q] = 1 if q >= k else 0
        nc.gpsimd.memset(tri[:], 1.0)
        nc.gpsimd.affine_select(
            out=tri[:], in_=tri[:], compare_op=mybir.AluOpType.is_ge,
            fill=0.0, base=0, pattern=[[1, 128]], channel_multiplier=-1,
        )
        ones1 = const.tile([1, 128], BF)
        nc.vector.memset(ones1[:], 1.0)

        # --- weights / biases in SBUF ---
        wq_sb = wpool.tile([128, NCH, DL], BF)
        wk_sb = wpool.tile([128, NCH, DL], BF)
        wv_sb = wpool.tile([128, NCH, DL], BF)
        nc.sync.dma_start(out=wq_sb[:], in_=wqT.ap().rearrange("(c p) m -> p c m", p=128))
        nc.sync.dma_start(out=wk_sb[:], in_=wkT.ap().rearrange("(c p) m -> p c m", p=128))
        nc.sync.dma_start(out=wv_sb[:], in_=wvT.ap().rearrange("(c p) m -> p c m", p=128))
        bq_sb = wpool.tile([DL, 1], F32)
        bk_sb = wpool.tile([DL, 1], F32)
        bvr_sb = wpool.tile([1, DL], BF)
        nc.sync.dma_start(out=bq_sb[:], in_=bq[:])
        nc.sync.dma_start(out=bk_sb[:], in_=bk[:])
        nc.sync.dma_start(out=bvr_sb[:], in_=bvr[:])
        wo_sb = wpool.tile([DL, D], BF)
        nc.sync.dma_start(out=wo_sb[:], in_=woT[:])

        # --- persistent activations ---
        qpT_sb = persist.tile([DL, BS], BF)   # [feat, tok]
        # k projection, zero-padded to K=128 per head so score matmuls use the
        # full PE contraction height (K=64 matmuls never trip the HAM clock
        # un-throttle; zero rows are free). kpz0 rows 0:64 = head-0 features,
        # rows 64:128 zero; kpz1 is the mirror. Both heads then share the full
        # qpT_sb[:, q] as the moving operand.
        kpz0 = persist.tile([128, BS], BF)
        kpz1 = persist.tile([128, BS], BF)
        nc.vector.memset(kpz0[64:128, :], 0.0)
        nc.vector.memset(kpz1[0:64, :], 0.0)
        # v projection, token-major, per head augmented with a ones column:
        # cols 0:64 head0, 64 ones, 65:129 head1, 129 ones
        vp_sb = persist.tile([128, BS // 128, 130], BF)
        nc.vector.memset(vp_sb[:, :, 64:65], 1.0)
        nc.vector.memset(vp_sb[:, :, 129:130], 1.0)
        oT_sb = persist.tile([DL, BS], BF)    # normalized attn output, [dim, tok]

        PT = 1024  # projection tile width (tokens)

        def proj_qk_tile(name, t):
            """One 1024-token tile of the q or k projection (transposed out)."""
            xT, w_sb, b_sb = {"q": (qT, wq_sb, bq_sb),
                              "k": (kT, wk_sb, bk_sb)}[name]
            ps = mm_ps.tile([128, 1024], F32, tag="mm")
            for c in range(NCH):
                x_tile = inp.tile([128, PT], BF, tag="xin")
                nc.sync.dma_start(
                    out=x_tile[:],
                    in_=xT[c * 128:(c + 1) * 128, t * PT:(t + 1) * PT])
                for j in range(PT // 512):
                    nc.tensor.matmul(
                        ps[:DL, j * 512:(j + 1) * 512], w_sb[:, c, :],
                        x_tile[:, j * 512:(j + 1) * 512],
                        start=(c == 0), stop=(c == NCH - 1))
            t_sl = slice(t * PT, (t + 1) * PT)
            if name == "q":
                nc.vector.tensor_scalar_add(qpT_sb[:, t_sl], ps[:DL, :], b_sb[:])
            else:
                nc.vector.tensor_scalar_add(
                    kpz0[0:64, t_sl], ps[0:64, :], b_sb[0:64])
                nc.vector.tensor_scalar_add(
                    kpz1[64:128, t_sl], ps[64:128, :], b_sb[64:128])

        def proj_v_tile(t):
            """1024 tokens of the v projection (token-major out, + bias),
            eight 128-token blocks packed into one psum tile."""
            vts = []
            for c in range(NCH):
                x_tile = vinp.tile([128, PT], BF, tag="vin")
                nc.gpsimd.dma_start(
                    out=x_tile[:],
                    in_=vT[c * 128:(c + 1) * 128, t * PT:(t + 1) * PT])
                vts.append(x_tile)
            ps = mm_ps.tile([128, 1024], F32, tag="mm")
            for g in range(PT // 128):
                for c in range(NCH):
                    nc.tensor.matmul(
                        ps[:, g * 128:g * 128 + DL],
                        vts[c][:, g * 128:(g + 1) * 128], wv_sb[:, c, :],
                        start=(c == 0), stop=False)
                nc.tensor.matmul(ps[:, g * 128:g * 128 + DL], ones1[:],
                                 bvr_sb[:], start=False, stop=True)
                tb = t * (PT // 128) + g
                nc.vector.tensor_copy(vp_sb[:, tb, 0:64],
                                      ps[:, g * 128:g * 128 + 64])
                nc.vector.tensor_copy(vp_sb[:, tb, 65:129],
                                      ps[:, g * 128 + 64:(g + 1) * 128])

        def attention_qtile(b, qt):
            """Causal attention for one q-tile, both heads software-pipelined:
            PE alternates between the two heads' independent score/PV chains so
            exp latency on ScalarE never drains the PE queue."""
            col0 = b * S
            qb0 = qt * (QT // 128)  # first q-block index (batch-local)
            q_sl = slice(col0 + qt * QT, col0 + (qt + 1) * QT)

            # step plan shared by both heads
            steps = []
            fulls = list(range(qb0))
            i = 0
            while i + 1 < len(fulls):
                steps.append(("pair", fulls[i], fulls[i + 1]))
                i += 2
            if i < len(fulls):
                steps.append(("single", fulls[i]))
            for s_i in range(QT // 128):
                steps.append(("diag", qb0 + s_i, s_i))

            pv0 = pv_ps_pool.tile([65, QT], F32, tag="pv")
            pv1 = pv_ps_pool.tile([65, QT], F32, tag="pv")
            pvs = [pv0, pv1]
            first = [True, True]

            def emit_scores(step):
                items = []  # (h, kb, exp_ap, n, qoff)
                for h, kpz in ((0, kpz0), (1, kpz1)):
                    ps = mm_ps.tile([128, 1024], F32, tag="mm")
                    e = work.tile([128, 1024], BF, tag="exp")
                    if step[0] == "pair":
                        for j, kb in enumerate(step[1:]):
                            k_sl = slice(col0 + kb * 128, col0 + (kb + 1) * 128)
                            nc.tensor.matmul(
                                ps[:, j * QT:(j + 1) * QT],
                                kpz[:, k_sl], qpT_sb[:, q_sl],
                                start=True, stop=True)
                        nc.scalar.activation(e[:], ps[:], EXP, scale=0.125)
                        items.append((h, step[1], e[:, 0:QT], QT, 0))
                        items.append((h, step[2], e[:, QT:2 * QT], QT, 0))
                    elif step[0] == "single":
                        kb = step[1]
                        k_sl = slice(col0 + kb * 128, col0 + (kb + 1) * 128)
                        nc.tensor.matmul(
                            ps[:, 0:QT], kpz[:, k_sl],
                            qpT_sb[:, q_sl], start=True, stop=True)
                        nc.scalar.activation(e[:, 0:QT], ps[:, 0:QT], EXP,
                                             scale=0.125)
                        items.append((h, kb, e[:, 0:QT], QT, 0))
                    else:  # diag
                        kb, s_i = step[1], step[2]
                        n = QT - s_i * 128
                        qoff = s_i * 128
                        k_sl = slice(col0 + kb * 128, col0 + (kb + 1) * 128)
                        nc.tensor.matmul(
                            ps[:, 0:n], kpz[:, k_sl],
                            qpT_sb[:, q_sl.start + qoff:q_sl.stop],
                            start=True, stop=True)
                        nc.scalar.activation(e[:, 0:n], ps[:, 0:n], EXP,
                                             scale=0.125)
                        nc.vector.tensor_tensor(out=e[:, 0:128], in0=e[:, 0:128],
                                                in1=tri[:], op=MULT)
                        items.append((h, kb, e[:, 0:n], n, qoff))
                return items

            def flush(items, last):
                for h, kb, exp_ap, n, qoff in items:
                    vb = b * NB + kb
                    vaug = vp_sb[:, vb, 0:65] if h == 0 else vp_sb[:, vb, 65:130]
                    nc.tensor.matmul(pvs[h][:, qoff:qoff + n], vaug, exp_ap,
                                     start=first[h], stop=last)
                    first[h] = False

            pending = []
            for idx, step in enumerate(steps):
                items = emit_scores(step)
                flush(pending, last=False)
                pending = items
            flush(pending, last=True)

            # normalize both heads: out = pv[0:64] * broadcast(1 / pv[64])
            for h in (0, 1):
                hs = h * DK
                pv = pvs[h]
                rec = evict.tile([1, QT], F32, tag="rec")
                rec_bf = evict.tile([1, QT], BF, tag="recbf")
                rs_sb = evict.tile([1, QT], F32, tag="rs")
                nc.vector.tensor_copy(rs_sb[:], pv[64:65, :])
                nc.vector.reciprocal_approx_fast(out=rec[:], in_=rs_sb[:])
                nc.vector.tensor_copy(rec_bf[:], rec[:])
                rb = rb_ps_pool.tile([64, QT], F32, tag="rb")
                nc.tensor.matmul(rb[:], ones1[:, 0:64], rec_bf[:],
                                 start=True, stop=True)
                raw = evict.tile([64, QT], BF, tag="raw")
                nc.vector.tensor_copy(raw[:], pv[0:64, :])
                nc.vector.tensor_tensor(out=oT_sb[hs:hs + DK, q_sl], in0=raw[:],
                                        in1=rb[:], op=MULT)

        def wo_block(tb):
            """Output projection partial for one 128-token block."""
            ps = mm_ps.tile([128, 1024], F32, tag="mm")
            lhsT = oT_sb[:, tb * 128:(tb + 1) * 128]
            nc.tensor.matmul(ps[:, 0:512], lhsT, wo_sb[:, 0:512],
                             start=True, stop=True)
            nc.tensor.matmul(ps[:, 512:1024], lhsT, wo_sb[:, 512:1024],
                             start=True, stop=True)
            o = evict.tile([128, 1024], F32, tag="out")
            nc.vector.tensor_copy(o[:], ps[:])
            nc.sync.dma_start(out=out[tb * 128:(tb + 1) * 128, :], in_=o[:])

        tiles_per_b = S // PT  # proj tiles per batch

        def proj_tensor(name, b):
            for t in range(b * tiles_per_b, (b + 1) * tiles_per_b):
                if name == "v":
                    proj_v_tile(t)
                else:
                    proj_qk_tile(name, t)

        def wo_blocks(b, qt):
            for i in range(QT // 128):
                wo_block(b * NB + qt * (QT // 128) + i)

        # Batch-0 projections, then attention with wo stores emitted per
        # q-tile; batch-1 projections are woven into batch-0's attention loop
        # so their input DMAs stream while the PE crunches batch 0.
        proj_tensor("q", 0)
        proj_tensor("k", 0)
        proj_tensor("v", 0)
        for qt in range(S // QT):
            attention_qtile(0, qt)
            wo_blocks(0, qt)
            if qt == 0:
                proj_tensor("q", 1)
            elif qt == 1:
                proj_tensor("k", 1)
            elif qt == 2:
                proj_tensor("v", 1)
        for qt in range(S // QT):
            attention_qtile(1, qt)
            wo_blocks(1, qt)

    nc.compile()
    return nc


def _get_nc():
    if "nc" not in _compiled:
        _compiled["nc"] = _build()
    return _compiled["nc"]


def _numpy_fallback(q, k, v, mask, w_q, b_q, w_k, b_k, w_v, b_v, w_o, b_o):
    qp = q @ w_q.T + b_q
    kp = k @ w_k.T + b_k
    vp = v @ w_v.T + b_v
    qh = qp.reshape(B, S, H, DK).transpose(0, 2, 1, 3)
    kh = kp.reshape(B, S, H, DK).transpose(0, 2, 1, 3)
    vh = vp.reshape(B, S, H, DK).transpose(0, 2, 1, 3)
    out = np.empty((B, H, S, DK), np.float32)
    m = np.broadcast_to(mask, (1, 1, S, S))[0, 0]
    for b in range(B):
        for h in range(H):
            sc = qh[b, h] @ kh[b, h].T / np.sqrt(DK)
            sc = np.where(m == 0, -1e9, sc)
            sc -= sc.max(axis=-1, keepdims=True)
            e = np.exp(sc)
            a = e / e.sum(axis=-1, keepdims=True)
            out[b, h] = a @ vh[b, h]
    out = out.transpose(0, 2, 1, 3).reshape(B, S, D)
    return (out @ w_o.T + b_o).astype(np.float32)


def kernel(q, k, v, mask, w_q, b_q, w_k, b_k, w_v, b_v, w_o, b_o):
    m2 = np.broadcast_to(np.asarray(mask), (1, 1, S, S))[0, 0]
    if not np.array_equal(m2 != 0, np.tril(np.ones((S, S), bool))):
        return _numpy_fallback(q, k, v, mask, w_q, b_q, w_k, b_k,
                               w_v, b_v, w_o, b_o)

    from concourse.bass_utils import run_bass_kernel_spmd

    nc = _get_nc()

    def t2d(x):  # [B,S,D] -> [D, B*S] bf16
        return np.ascontiguousarray(
            np.asarray(x, np.float32).reshape(BS, D).T).astype(BF16)

    qT, kT, vT = t2d(q), t2d(k), t2d(v)
    in_maps = []
    for c in range(NCORES):
        r = slice(c * DL, (c + 1) * DL)
        in_maps.append({
            "qT": qT, "kT": kT, "vT": vT,
            "wqT": np.ascontiguousarray(np.asarray(w_q, np.float32)[r, :].T).astype(BF16),
            "wkT": np.ascontiguousarray(np.asarray(w_k, np.float32)[r, :].T).astype(BF16),
            "wvT": np.ascontiguousarray(np.asarray(w_v, np.float32)[r, :].T).astype(BF16),
            "bq": np.asarray(b_q, np.float32)[r].reshape(DL, 1).copy(),
            "bk": np.asarray(b_k, np.float32)[r].reshape(DL, 1).copy(),
            "bvr": np.asarray(b_v, np.float32)[r].reshape(1, DL).astype(BF16),
            "woT": np.ascontiguousarray(np.asarray(w_o, np.float32)[:, r].T).astype(BF16),
        })

    res = run_bass_kernel_spmd(nc, in_maps, list(range(NCORES)))
    total = np.zeros((BS, D), np.float32)
    for c in range(NCORES):
        total += res.results[c]["out"]
    total += np.asarray(b_o, np.float32)[None, :]
    return total.reshape(B, S, D)
